# revision 70
# baseline (speedup 1.0000x reference)
"""Multi-head self-attention (BERT-style) Trainium2 kernel.

Sharding: 8 cores = 2 batches x 4 head-groups (3 heads each).
Per core (batch b, heads h0..h2):
  Q^T/K^T = W^T X^T per head, duplicated across both partition halves
            (score matmuls contract K=128; the 2x is folded into scale —
            full-tile matmuls stream ~2.4x faster per instruction than
            tile_position K=64 matmuls on this hardware)
  V       = X Wv (+bias) with ones-column per head (denominator trick)
  scores  = K-tile x Q^T -> [128 keys, 1024 q] PSUM tiles,
            exp on ScalarE with attention-mask as per-partition bias,
            ONE activation instruction per 1024 q-columns
  ctx/denom via PV matmuls accumulating [65, 512] PSUM ctx quarters
  normalize: reciprocal of denom row -> partition_broadcast -> mult
  out partial = ctx01^T Wo[0:128] + ctx2d^T Wo2dup (rows pre-halved)
Host sums the 4 partials per batch and adds bo.

Schedule: phases run half-major — (h2,0),(h1,0),(h0,0),(h2,1),(h1,1),
(h0,1) — so head 2 (whose Q/K come packed per wb2 tile) needs only one
projection tile type in the lead, and the last phase is head 0, whose
normalize writes ctx01 rows 0:64 directly (no trailing dup DMA). xt is
s-quarter-major, split across the sync/gpsimd/scalar DMA queues (each
queue carries only ~1/3 of aggregate bandwidth); sync carries only xt q0
then becomes the clean low-latency channel for the qd/kd partition-dup
DMAs. Dummy zero matmuls hold the PE clock at full pstate through the
DMA lead-in and the tail normalize gap (the clock ramps over ~3us of
continuous execution and resets on idle). PV pops trail scores by
PV_LAG=4, trimmed to 2 at phase boundaries with the leftovers popped in
the next phase's first c's so each phase's normalize chain completes
before the next phase's first PV hits the 2-deep ctx-slot ring. Fillers:
V + remaining Q/K projections spread over phases 0-2; output projection
q0-7 rides phases 3-5 (emitted ahead of trailing normalizes to dodge
tile-granular false deps); the tail after the last exp is only: drain,
both-quarter normalize, outproj q8-15.
PSUM: 3x[128,1024] work slots + 2x[128,512] ctx slots = 8 banks.
"""

import sys

sys.path.insert(0, "/opt/trn_rl_repo")

from contextlib import ExitStack

import numpy as np

import concourse.bass as bass
import concourse.mybir as mybir
import concourse.tile as tile
from concourse import bacc
from concourse.bass_utils import run_bass_kernel_spmd

F16 = mybir.dt.float16
F32 = mybir.dt.float32

H = 768
NH = 12
HD = 64
B = 2
S = 2048
HC = H // 128  # 6 h-chunks of 128
KT = S // 128  # 16 k-tiles of 128
QT = 4  # 4 s-quarters of 512
D3 = 3 * HD  # 192 cols per core
N_CORES = 8


def build_kernel():
    nc = bacc.Bacc(
        "TRN2",
        target_bir_lowering=False,
        debug=False,
        enable_asserts=False,
        num_devices=N_CORES,
    )

    # All inputs partition-major so every DMA descriptor carries a
    # multi-KB partition line (DMA engines pay ~220ns fixed per
    # descriptor). xt is s-quarter-major: [128, QT, HC, 512].
    xt = nc.dram_tensor("xt", [128, QT * HC * 512], F16, kind="ExternalInput")
    wqk = nc.dram_tensor("wqk", [128, 3 * HC * 128], F16, kind="ExternalInput")
    wv = nc.dram_tensor("wv", [128, HC * D3], F16, kind="ExternalInput")
    wo = nc.dram_tensor("wo", [128, 2 * H], F16, kind="ExternalInput")
    bqkm = nc.dram_tensor("bqkm", [128, 20], F32, kind="ExternalInput")
    bv = nc.dram_tensor("bv", [1, D3], F16, kind="ExternalInput")
    out = nc.dram_tensor("out", [S, H], F16, kind="ExternalOutput")

    with tile.TileContext(nc) as tc:
        _emit(tc, xt, wqk, wv, wo, bqkm, bv, out)

    nc.compile()
    return nc


def _emit(tc, xt, wqk, wv, wo, bqkm, bv, out):
    nc = tc.nc
    ADD = mybir.AluOpType.add
    MULT = mybir.AluOpType.mult
    EXP = mybir.ActivationFunctionType.Exp

    with ExitStack() as stack:
        persist = stack.enter_context(tc.tile_pool(name="persist", bufs=1))

        # ---- constant / persistent SBUF tiles (blob DMAs + views) ----
        xt_sb = persist.tile([128, QT, HC, 512], F16)
        wqk_sb = persist.tile([128, 3, HC, 128], F16)
        wb2_sb, wq_sb, wk_sb = wqk_sb[:, 0], wqk_sb[:, 1], wqk_sb[:, 2]
        wv_sb = persist.tile([128, HC, D3], F16)
        woo_sb = persist.tile([128, 2 * H], F16)
        wo_sb, wo2d = woo_sb[:, 0:H], woo_sb[:, H:]
        bqkm_sb = persist.tile([128, 20], F32)
        bq_sb, bk_sb = bqkm_sb[:, 0:2], bqkm_sb[:, 2:4]
        mask_sb = bqkm_sb[:, 4:20]
        bv_sb = persist.tile([1, D3], F16)

        # Per-queue DMA bandwidth is only ~1/3 of the per-core HBM rate, so
        # the lead-gating transfers spread across four queues: xt q0 on
        # sync, q1 on gpsimd, q2 on vector (issued before its memsets),
        # q3 + weights on scalar. wb2 (192KB) gates the bgroup lead.
        def xt_quarter(q):
            return (
                xt_sb[:, q].rearrange("p c s -> p (c s)"),
                xt.ap()[:, q * HC * 512 : (q + 1) * HC * 512],
            )

        # xt moves as whole 6KB/partition quarters: 3KB descriptors are
        # bound by the ~220ns fixed cost per descriptor (~62% efficiency),
        # 6KB rows run at full rate. Queues: q0+q2 then wqk/woo on sync,
        # q1+q3 on gpsimd, the small weights on scalar — scalar clears by
        # ~12us and carries phase-0's dup DMAs; sync clears later and
        # carries phase-1/2's dups (scalar would put those issues in the
        # middle of the exp stream on the by-then-busy ACT engine).
        nc.sync.dma_start(*xt_quarter(0))
        nc.gpsimd.dma_start(*xt_quarter(1))
        nc.scalar.dma_start(bv_sb[:], bv.ap())
        nc.scalar.dma_start(
            wqk_sb[:, 0].rearrange("p c d -> p (c d)"), wqk.ap()[:, 0 : HC * 128]
        )
        nc.sync.dma_start(*xt_quarter(2))
        nc.gpsimd.dma_start(*xt_quarter(3))
        nc.scalar.dma_start(bqkm_sb[:], bqkm.ap())
        nc.scalar.dma_start(wv_sb[:].rearrange("p c d -> p (c d)"), wv.ap())
        nc.sync.dma_start(
            wqk_sb[:, 1:3].rearrange("p w c d -> p (w c d)"),
            wqk.ap()[:, HC * 128 :],
        )
        nc.sync.dma_start(woo_sb[:], wo.ap())
        bv_bc = persist.tile([128, D3], F16)
        nc.gpsimd.partition_broadcast(bv_bc[:], bv_sb[:])
        # PE keep-warm fodder: the PE clock ramps to full speed only after
        # ~3us of continuous execution and drops back during idle gaps, so
        # dummy matmuls (zero inputs, discarded output) hold the clock up
        # through the DMA lead-in and the tail normalize gap.
        warmpe = persist.tile([128, 512], F16)
        nc.vector.memset(warmpe[:], 0.0)
        # warm the ACT exp table during the DMA lead-in
        warm = persist.tile([1, 8], F32)
        nc.vector.memset(warm[:], 0.0)
        nc.scalar.activation(warm[:], warm[:], EXP)

        def emit_pe_warm(n):
            for _ in range(n):
                wps = work.tile([128, 512], F32, tag="wk", name="wps")
                nc.tensor.matmul(
                    wps[0:64, :],
                    lhsT=warmpe[:, 0:64],
                    rhs=warmpe[:],
                    start=True,
                    stop=True,
                )

        # Q^T/K^T per head, duplicated across both partition halves
        qd = [persist.tile([128, S], F16, name=f"qd{h}") for h in range(3)]
        kd = [persist.tile([128, S], F16, name=f"kd{h}") for h in range(3)]
        # V: [k, 3*(64+1)] with a ones column per head (col 64 of each 65)
        v_sb = persist.tile([128, KT, 3 * 65], F16)
        for h in range(3):
            nc.vector.memset(
                v_sb[:].rearrange("p k (h x) -> p k h x", x=65)[:, :, h, 64:65], 1.0
            )
        # normalized context: heads 0,1 stacked; head 2 duplicated
        ctx01 = persist.tile([128, S], F16)
        ctx2d = persist.tile([128, S], F16)
        ctx_tmp = persist.tile([64, S], F16)

        # ---- PSUM: 3x [128,1024] work slots (12KB) + 2x [128,512] ctx
        # slots (4KB) = exactly 8 banks.
        ctx_pool = tc.alloc_tile_pool(name="ctx_ps", bufs=2, space="PSUM")
        work = tc.alloc_tile_pool(name="work", bufs=3, space="PSUM")
        p_pool = stack.enter_context(tc.tile_pool(name="p_sb", bufs=8))
        norm_pool = stack.enter_context(tc.tile_pool(name="norm", bufs=4))
        ob_pool = stack.enter_context(tc.tile_pool(name="ob", bufs=6))

        def emit_qk(w_sb, dst, b_sb, qt, bgroup, psum=None, dq=None):
            """One [128, 512] projection tile + bias + partition-dup DMAs."""
            dq = dq or nc.sync
            qs = slice(qt * 512, (qt + 1) * 512)
            pq = psum if psum is not None else work.tile(
                [128, 512], F32, tag="wk", name="pq"
            )
            for hc in range(HC):
                nc.tensor.matmul(
                    pq[:],
                    lhsT=w_sb[:, hc, :],
                    rhs=xt_sb[:, qt, hc, :],
                    start=(hc == 0),
                    stop=(hc == HC - 1),
                )
            if bgroup:
                # rows 0:64 = Q2, rows 64:128 = K2 (w_sb is [Wq2 | Wk2])
                nc.vector.tensor_scalar(
                    qd[2][0:64, qs], pq[0:64, :], b_sb[0:64, 1:2], None, ADD
                )
                nc.vector.tensor_scalar(
                    kd[2][64:128, qs], pq[64:128, :], b_sb[64:128, 1:2], None, ADD
                )
                dq.dma_start(qd[2][64:128, qs], qd[2][0:64, qs])
                dq.dma_start(kd[2][0:64, qs], kd[2][64:128, qs])
            else:
                nc.vector.tensor_scalar(
                    dst[0][0:64, qs], pq[0:64, :], b_sb[0:64, 0:1], None, ADD
                )
                nc.vector.tensor_scalar(
                    dst[1][64:128, qs], pq[64:128, :], b_sb[64:128, 0:1], None, ADD
                )
                dq.dma_start(dst[0][64:128, qs], dst[0][0:64, qs])
                dq.dma_start(dst[1][0:64, qs], dst[1][64:128, qs])

        def emit_v(kt):
            q, r = divmod(kt, 4)
            pv = work.tile([128, D3], F32, tag="wk", name="pv")
            for hc in range(HC):
                nc.tensor.matmul(
                    pv[:],
                    lhsT=xt_sb[:, q, hc, r * 128 : (r + 1) * 128],
                    rhs=wv_sb[:, hc, :],
                    start=(hc == 0),
                    stop=(hc == HC - 1),
                )
            nc.vector.tensor_tensor(
                v_sb[:].rearrange("p k (h x) -> p k h x", x=65)[:, kt, :, 0:64],
                pv[:].rearrange("p (h x) -> p h x", x=64),
                bv_bc[:].rearrange("p (h x) -> p h x", x=64),
                ADD,
            )

        # ---- score/exp/PV pipeline ----
        # pv_q holds exp'd tiles; each pop issues the 2 PV matmuls for one
        # [128,1024] prob tile. The lag keeps the in-order PE from stalling
        # on a not-yet-finished exp, and carries ctx WAR slack across
        # phase boundaries (ctx ring bufs=2).
        pv_q = []
        PV_LAG = 3

        def pop_pv(tail=False):
            h, half, c, cq, pt = pv_q.pop(0)
            for j in range(2):
                nc.tensor.matmul(
                    cq[j][:],
                    lhsT=v_sb[:, c, h * 65 : (h + 1) * 65],
                    rhs=pt[:, j * 512 : (j + 1) * 512],
                    start=(c == 0),
                    stop=(c == KT - 1),
                )
            # the final phase (h0, half1) normalizes per-quarter in the
            # tail instead; every other phase normalizes on its last pop
            if c == KT - 1 and not (h == 0 and half == 1):
                emit_normalize(h, half, cq)

        def emit_score_tile(h, half, c, cq, tail=False):
            """[128 keys, 1024 q] scores -> exp -> queue PV."""
            ks = slice(c * 128, (c + 1) * 128)
            sc = work.tile([128, 1024], F32, tag="wk", name="sc")
            for j in range(2):
                qs = slice(half * 1024 + j * 512, half * 1024 + (j + 1) * 512)
                nc.tensor.matmul(
                    sc[:, j * 512 : (j + 1) * 512],
                    lhsT=kd[h][:, ks],
                    rhs=qd[h][:, qs],
                    start=True,
                    stop=True,
                )
            pt = p_pool.tile([128, 1024], F16, tag="pt")
            nc.scalar.activation(
                pt[:], sc[:], EXP, bias=mask_sb[:, c : c + 1], scale=1.0
            )
            pv_q.append((h, half, c, cq, pt))
            # mostly-lazy drain: trim to 2 at the boundary and pop the two
            # leftovers in the next phase's first c's, so the normalize
            # chain (which gates the ctx-slot WAR for the NEXT phase's
            # first PV — the ctx ring is only 2 deep) starts early without
            # the PE ever waiting on a boundary exp that isn't done yet
            if c == KT - 1:
                while len(pv_q) > 2:
                    pop_pv(tail)
            elif c == 0:
                # pop the boundary leftovers now: their exps are (nearly)
                # done, and the previous phase's normalize then has ~4
                # tiles to clear the ctx-slot WAR before this phase's
                # first PV pops at c=4
                while len(pv_q) > 1:
                    pop_pv(tail)
            elif len(pv_q) > PV_LAG:
                pop_pv(tail)

        def emit_normalize(h, half, cq):
            """denom copy -> recip -> broadcast -> mult, stage-major over
            the half's two 512-col ctx quarters."""
            base = half * 1024
            if h == 0:
                dst = ctx01[0:64, :]
            elif h == 1:
                dst = ctx_tmp[:]
            else:
                dst = ctx2d[0:64, :]
            denoms, recips, rbcs = [], [], []
            for j in range(2):
                denom = norm_pool.tile([1, 512], F32, tag="denom")
                nc.vector.tensor_copy(denom[:], cq[j][64:65, :])
                denoms.append(denom)
            for j in range(2):
                recip = norm_pool.tile([1, 512], F32, tag="recip")
                nc.vector.reciprocal_approx_fast(recip[:], denoms[j][:])
                recips.append(recip)
            for j in range(2):
                rbc = norm_pool.tile([64, 512], F32, tag="rbc")
                nc.gpsimd.partition_broadcast(rbc[:], recips[j][:])
                rbcs.append(rbc)
            for j in range(2):
                dcols = slice(base + j * 512, base + (j + 1) * 512)
                nc.vector.tensor_tensor(
                    dst[:, dcols], cq[j][0:64, :], rbcs[j][:], MULT
                )
            if h == 1:
                nc.gpsimd.dma_start(
                    ctx01[64:128, base : base + 1024], ctx_tmp[:, base : base + 1024]
                )
            elif h == 2:
                nc.gpsimd.dma_start(
                    ctx2d[64:128, base : base + 1024], ctx2d[0:64, base : base + 1024]
                )

        def emit_norm_tail(half, cq):
            """Last-phase normalize, both quarters with minimal latency to
            the first mult: per-quarter copy+recip chains on DVE (ACT is
            still draining the final exps), broadcasts pipelined on GpSimd."""
            base = half * 1024
            rbcs = []
            for j in range(2):
                denom = norm_pool.tile([1, 512], F32, tag="denom")
                nc.vector.tensor_copy(denom[:], cq[j][64:65, :])
                recip = norm_pool.tile([1, 512], F32, tag="recip")
                nc.vector.reciprocal_approx_fast(recip[:], denom[:])
                rbc = norm_pool.tile([64, 512], F32, tag="rbc")
                nc.gpsimd.partition_broadcast(rbc[:], recip[:])
                rbcs.append(rbc)
            for j in range(2):
                dcols = slice(base + j * 512, base + (j + 1) * 512)
                nc.vector.tensor_tensor(
                    ctx01[0:64, dcols], cq[j][0:64, :], rbcs[j][:], MULT
                )

        def emit_outproj(qt, po, cast_eng, dma_eng):
            """po = ctx01^T Wo01 + ctx2d^T Wo2dup for one 128-row q-tile."""
            qs = slice(qt * 128, (qt + 1) * 128)
            # matmul dest is capped at 512 fp32 elements (one PSUM bank)
            for ns, ne in ((0, 512), (512, 768)):
                nc.tensor.matmul(
                    po[:, ns:ne],
                    lhsT=ctx01[:, qs],
                    rhs=wo_sb[:, ns:ne],
                    start=True,
                    stop=False,
                )
                nc.tensor.matmul(
                    po[:, ns:ne],
                    lhsT=ctx2d[:, qs],
                    rhs=wo2d[:, ns:ne],
                    start=False,
                    stop=True,
                )
            ob = ob_pool.tile([128, H], F16, tag="ob")
            if cast_eng == "v":
                nc.vector.tensor_copy(ob[:], po[:])
            else:
                nc.scalar.copy(ob[:], po[:])
            dma_eng.dma_start(out.ap()[qs, :], ob[:])

        # ---------------- emission schedule ----------------
        # Lead-in: dummy matmuls ramp the PE clock while the wb2/xt DMAs
        # land; head-2's Q/K come packed per wb2 tile (rows 0:64 = Q2,
        # 64:128 = K2), so two bgroup tiles cover all scores of phase 0's
        # first half. Gated on xt quarters 0/1 + the wb2 blob.
        emit_pe_warm(14)
        pb0 = work.tile([128, 512], F32, tag="wk", name="pb0")
        emit_qk(wb2_sb, None, bq_sb, 0, True, psum=pb0, dq=nc.scalar)
        pb1 = ctx_pool.tile([128, 512], F32, tag="ctx", name="pb1")
        emit_qk(wb2_sb, None, bq_sb, 1, True, psum=pb1, dq=nc.scalar)

        def ctx_quarters(name):
            return (
                ctx_pool.tile([65, 512], F32, tag="ctx", name=f"{name}q0"),
                ctx_pool.tile([65, 512], F32, tag="ctx", name=f"{name}q1"),
            )

        # fillers[phase][c] = list of thunks to emit before that score tile.
        # Phase-0 dups ride scalar (clear by ~12us, ACT still idle there);
        # phase-1/2 dups ride sync (clear by then; scalar's ACT engine is
        # busy with exps by phase 1 and a pseudo-DMA issue would wedge
        # ~600ns into the exp stream).
        def qk_thunk(w, dst, b, qt, bgroup=False, dq=None):
            return lambda: emit_qk(w, dst, b, qt, bgroup, dq=dq)

        fillers = {
            0: {
                5: [qk_thunk(wb2_sb, None, bq_sb, 2, True, dq=nc.scalar)],
                8: [qk_thunk(wk_sb, kd, bk_sb, 0, dq=nc.scalar)],
                9: [qk_thunk(wb2_sb, None, bq_sb, 3, True, dq=nc.scalar)],
                10: [qk_thunk(wq_sb, qd, bq_sb, 0, dq=nc.scalar)],
                # Q1 early enough that its bias+dup chain completes before
                # phase 1's first score needs qd[1]
                12: [qk_thunk(wq_sb, qd, bq_sb, 1, dq=nc.scalar)],
            },
            1: {
                1: [qk_thunk(wk_sb, kd, bk_sb, 1)],
                5: [qk_thunk(wk_sb, kd, bk_sb, 2)],
                9: [qk_thunk(wk_sb, kd, bk_sb, 3)],
            },
            2: {
                3: [qk_thunk(wq_sb, qd, bq_sb, 2)],
                7: [qk_thunk(wq_sb, qd, bq_sb, 3)],
            },
        }

        phases = [(2, 0), (1, 0), (0, 0), (2, 1), (1, 1), (0, 1)]

        def outproj_thunk(qt, cast_eng, dma_eng):
            def f():
                po = work.tile([128, H], F32, tag="wk", name=f"po{qt}")
                emit_outproj(qt, po, cast_eng, dma_eng)

            return f

        # outproj q0-7 spread over phases 3-5 (phases 4-5 are otherwise
        # ACT-paced with PE slack); casts ride DVE to keep ACT on exps
        # ops early in phases 4-5 are emitted BEFORE that phase's trailing
        # normalize of the previous phase, so they dodge the tile-granular
        # false dependency on the ctx tiles
        op_sched = {
            3: {11: 0, 13: 1, 14: 2},
            4: {1: 3, 2: 4, 13: 5},
            5: {1: 6, 2: 7},
        }
        for pi, (h, half) in enumerate(phases):
            tail = pi == 5
            cq = ctx_quarters(f"c{h}{half}")
            for c in range(KT):
                for thunk in fillers.get(pi, {}).get(c, []):
                    thunk()
                emit_score_tile(h, half, c, cq, tail)
                if pi == 0:
                    emit_v(c)
                qt_op = op_sched.get(pi, {}).get(c)
                if qt_op is not None:
                    outproj_thunk(
                        qt_op, "v", nc.sync if qt_op % 2 == 0 else nc.scalar
                    )()
            if tail:
                while pv_q:
                    pop_pv(tail=True)
                # normalize both quarters up front; dummy matmuls bridge the
                # PE gap while the chain runs so the clock stays at full
                # speed for the outprojs
                emit_norm_tail(half, cq)
                emit_pe_warm(16)
                engs = (nc.sync, nc.scalar, nc.gpsimd)
                for i, qt in enumerate(range(8, KT)):
                    po = work.tile([128, H], F32, tag="wk", name=f"po{qt}")
                    emit_outproj(qt, po, "s" if i % 2 else "v", engs[i % 3])

        work.release()
        ctx_pool.release()


_NC_CACHE = None


def _get_nc():
    global _NC_CACHE
    if _NC_CACHE is None:
        _NC_CACHE = build_kernel()
    return _NC_CACHE


def _pack_w(w, ncols):
    """[768, ncols] -> [128, HC*ncols] with row p = concat_c w[c*128+p, :]."""
    return np.ascontiguousarray(
        w.reshape(HC, 128, ncols).transpose(1, 0, 2).reshape(128, HC * ncols)
    )


def make_in_maps(hidden_states, attention_mask, Wq, bq, Wk, bk, Wv, bv, Wo, bo):
    hidden_states = np.asarray(hidden_states, np.float32)
    attention_mask = np.asarray(attention_mask, np.float32)
    Wq = np.asarray(Wq, np.float32)
    Wk = np.asarray(Wk, np.float32)
    Wv = np.asarray(Wv, np.float32)
    Wo = np.asarray(Wo, np.float32)
    bq = np.asarray(bq, np.float32)
    bk = np.asarray(bk, np.float32)
    bv = np.asarray(bv, np.float32)

    scale = 0.5 / np.sqrt(np.float32(HD))  # extra 1/2: scores use dup-row K=128
    in_maps = []
    for core in range(N_CORES):
        b, g = divmod(core, 4)
        cols = slice(D3 * g, D3 * (g + 1))
        bq_s = (bq[cols] * scale).astype(np.float32)
        bk_s = bk[cols].astype(np.float32)
        bq_pack = np.zeros((2, 128), np.float32)
        bq_pack[0] = bq_s[0:128]
        bq_pack[1, 0:64] = bq_s[128:192]
        bq_pack[1, 64:128] = bk_s[128:192]
        bk_pack = np.zeros((2, 128), np.float32)
        bk_pack[0] = bk_s[0:128]

        # xt s-quarter-major: [128, QT, HC, 512]
        xtp = (
            np.ascontiguousarray(hidden_states[b].T)
            .astype(np.float16)
            .reshape(HC, 128, QT, 512)
            .transpose(1, 2, 0, 3)
            .reshape(128, QT * HC * 512)
        )
        wq_p = _pack_w((Wq[:, cols][:, 0:128] * scale).astype(np.float16), 128)
        wk_p = _pack_w(Wk[:, cols][:, 0:128].astype(np.float16), 128)
        wb2_p = _pack_w(
            np.concatenate(
                [Wq[:, cols][:, 128:192] * scale, Wk[:, cols][:, 128:192]], axis=1
            ).astype(np.float16),
            128,
        )
        wqk_p = np.concatenate([wb2_p, wq_p, wk_p], axis=1)
        wv_p = _pack_w(Wv[:, cols].astype(np.float16), D3)
        wo2h = (Wo[cols, :][128:192] * 0.5).astype(np.float16)
        woo = np.concatenate(
            [
                Wo[cols, :][0:128].astype(np.float16),
                np.concatenate([wo2h, wo2h], axis=0),
            ],
            axis=1,
        )
        bqkm_p = np.concatenate(
            [
                bq_pack.T,
                bk_pack.T,
                attention_mask[b, 0, 0, :].reshape(KT, 128).T,
            ],
            axis=1,
        ).astype(np.float32)
        in_maps.append(
            {
                "xt": np.ascontiguousarray(xtp),
                "wqk": np.ascontiguousarray(wqk_p),
                "wv": np.ascontiguousarray(wv_p),
                "wo": np.ascontiguousarray(woo),
                "bqkm": np.ascontiguousarray(bqkm_p),
                "bv": bv[cols].reshape(1, D3).astype(np.float16),
            }
        )
    return in_maps


def assemble_out(results, bo):
    out = np.zeros((B, S, H), np.float32)
    for core in range(N_CORES):
        b = core // 4
        out[b] += results[core]["out"].astype(np.float32)
    out += np.asarray(bo, np.float32)
    return out


def kernel(hidden_states, attention_mask, Wq, bq, Wk, bk, Wv, bv, Wo, bo):
    in_maps = make_in_maps(
        hidden_states, attention_mask, Wq, bq, Wk, bk, Wv, bv, Wo, bo
    )
    res = run_bass_kernel_spmd(_get_nc(), in_maps, list(range(N_CORES)))
    return assemble_out(res.results, bo)


# revision 71
# speedup vs baseline: 1.0111x; 1.0111x over previous
"""Multi-head self-attention (BERT-style) Trainium2 kernel.

Sharding: 8 cores = 2 batches x 4 head-groups (3 heads each).
Per core (batch b, heads h0..h2):
  Q^T/K^T = W^T X^T per head, duplicated across both partition halves
            (score matmuls contract K=128; the 2x is folded into scale —
            full-tile matmuls stream ~2.4x faster per instruction than
            tile_position K=64 matmuls on this hardware)
  V       = X Wv (+bias) with ones-column per head (denominator trick)
  scores  = K-tile x Q^T -> [128 keys, 1024 q] PSUM tiles,
            exp on ScalarE with attention-mask as per-partition bias,
            ONE activation instruction per 1024 q-columns
  ctx/denom via PV matmuls accumulating [65, 512] PSUM ctx quarters
  normalize: reciprocal of denom row -> partition_broadcast -> mult
  out partial = ctx01^T Wo[0:128] + ctx2d^T Wo2dup (rows pre-halved)
Host sums the 4 partials per batch and adds bo.

Schedule: phases run half-major — (h2,0),(h1,0),(h0,0),(h2,1),(h1,1),
(h0,1) — so head 2 (whose Q/K come packed per wb2 tile) needs only one
projection tile type in the lead, and the last phase is head 0, whose
normalize writes ctx01 rows 0:64 directly (no trailing dup DMA). xt is
s-quarter-major, split across the sync/gpsimd/scalar DMA queues (each
queue carries only ~1/3 of aggregate bandwidth); sync carries only xt q0
then becomes the clean low-latency channel for the qd/kd partition-dup
DMAs. Dummy zero matmuls hold the PE clock at full pstate through the
DMA lead-in and the tail normalize gap (the clock ramps over ~3us of
continuous execution and resets on idle). PV pops trail scores by
PV_LAG=4, trimmed to 2 at phase boundaries with the leftovers popped in
the next phase's first c's so each phase's normalize chain completes
before the next phase's first PV hits the 2-deep ctx-slot ring. Fillers:
V + remaining Q/K projections spread over phases 0-2; output projection
q0-7 rides phases 3-5 (emitted ahead of trailing normalizes to dodge
tile-granular false deps); the tail after the last exp is only: drain,
both-quarter normalize, outproj q8-15.
PSUM: 3x[128,1024] work slots + 2x[128,512] ctx slots = 8 banks.
"""

import sys

sys.path.insert(0, "/opt/trn_rl_repo")

from contextlib import ExitStack

import numpy as np

import concourse.bass as bass
import concourse.mybir as mybir
import concourse.tile as tile
from concourse import bacc
from concourse.bass_utils import run_bass_kernel_spmd

F16 = mybir.dt.float16
F32 = mybir.dt.float32

H = 768
NH = 12
HD = 64
B = 2
S = 2048
HC = H // 128  # 6 h-chunks of 128
KT = S // 128  # 16 k-tiles of 128
QT = 4  # 4 s-quarters of 512
D3 = 3 * HD  # 192 cols per core
N_CORES = 8


def build_kernel():
    nc = bacc.Bacc(
        "TRN2",
        target_bir_lowering=False,
        debug=False,
        enable_asserts=False,
        num_devices=N_CORES,
    )

    # All inputs partition-major so every DMA descriptor carries a
    # multi-KB partition line (DMA engines pay ~220ns fixed per
    # descriptor). xt is s-quarter-major: [128, QT, HC, 512].
    xt = nc.dram_tensor("xt", [128, QT * HC * 512], F16, kind="ExternalInput")
    wqk = nc.dram_tensor("wqk", [128, 3 * HC * 128], F16, kind="ExternalInput")
    wv = nc.dram_tensor("wv", [128, HC * D3], F16, kind="ExternalInput")
    wo = nc.dram_tensor("wo", [128, 2 * H], F16, kind="ExternalInput")
    bqkm = nc.dram_tensor("bqkm", [128, 20], F32, kind="ExternalInput")
    bv = nc.dram_tensor("bv", [1, D3], F16, kind="ExternalInput")
    out = nc.dram_tensor("out", [S, H], F16, kind="ExternalOutput")

    with tile.TileContext(nc) as tc:
        _emit(tc, xt, wqk, wv, wo, bqkm, bv, out)

    nc.compile()
    return nc


def _emit(tc, xt, wqk, wv, wo, bqkm, bv, out):
    nc = tc.nc
    ADD = mybir.AluOpType.add
    MULT = mybir.AluOpType.mult
    EXP = mybir.ActivationFunctionType.Exp

    with ExitStack() as stack:
        persist = stack.enter_context(tc.tile_pool(name="persist", bufs=1))

        # ---- constant / persistent SBUF tiles (blob DMAs + views) ----
        xt_sb = persist.tile([128, QT, HC, 512], F16)
        wqk_sb = persist.tile([128, 3, HC, 128], F16)
        wb2_sb, wq_sb, wk_sb = wqk_sb[:, 0], wqk_sb[:, 1], wqk_sb[:, 2]
        wv_sb = persist.tile([128, HC, D3], F16)
        woo_sb = persist.tile([128, 2 * H], F16)
        wo_sb, wo2d = woo_sb[:, 0:H], woo_sb[:, H:]
        bqkm_sb = persist.tile([128, 20], F32)
        bq_sb, bk_sb = bqkm_sb[:, 0:2], bqkm_sb[:, 2:4]
        mask_sb = bqkm_sb[:, 4:20]
        bv_sb = persist.tile([1, D3], F16)

        # Per-queue DMA bandwidth is only ~1/3 of the per-core HBM rate, so
        # the lead-gating transfers spread across four queues: xt q0 on
        # sync, q1 on gpsimd, q2 on vector (issued before its memsets),
        # q3 + weights on scalar. wb2 (192KB) gates the bgroup lead.
        def xt_quarter(q):
            return (
                xt_sb[:, q].rearrange("p c s -> p (c s)"),
                xt.ap()[:, q * HC * 512 : (q + 1) * HC * 512],
            )

        # xt moves as whole 6KB/partition quarters: 3KB descriptors are
        # bound by the ~220ns fixed cost per descriptor (~62% efficiency),
        # 6KB rows run at full rate. Queues: q0+q2 then wqk/woo on sync,
        # q1+q3 on gpsimd, the small weights on scalar — scalar clears by
        # ~12us and carries phase-0's dup DMAs; sync clears later and
        # carries phase-1/2's dups (scalar would put those issues in the
        # middle of the exp stream on the by-then-busy ACT engine).
        nc.sync.dma_start(*xt_quarter(0))
        nc.gpsimd.dma_start(*xt_quarter(1))
        nc.scalar.dma_start(bv_sb[:], bv.ap())
        nc.scalar.dma_start(
            wqk_sb[:, 0].rearrange("p c d -> p (c d)"), wqk.ap()[:, 0 : HC * 128]
        )
        nc.sync.dma_start(*xt_quarter(2))
        nc.gpsimd.dma_start(*xt_quarter(3))
        nc.scalar.dma_start(bqkm_sb[:], bqkm.ap())
        nc.scalar.dma_start(wv_sb[:].rearrange("p c d -> p (c d)"), wv.ap())
        nc.sync.dma_start(
            wqk_sb[:, 1:3].rearrange("p w c d -> p (w c d)"),
            wqk.ap()[:, HC * 128 :],
        )
        nc.sync.dma_start(woo_sb[:], wo.ap())
        bv_bc = persist.tile([128, D3], F16)
        nc.gpsimd.partition_broadcast(bv_bc[:], bv_sb[:])
        # PE keep-warm fodder: the PE clock ramps to full speed only after
        # ~3us of continuous execution and drops back during idle gaps, so
        # dummy matmuls (zero inputs, discarded output) hold the clock up
        # through the DMA lead-in and the tail normalize gap.
        warmpe = persist.tile([128, 512], F16)
        nc.vector.memset(warmpe[:], 0.0)
        # warm the ACT exp table during the DMA lead-in
        warm = persist.tile([1, 8], F32)
        nc.vector.memset(warm[:], 0.0)
        nc.scalar.activation(warm[:], warm[:], EXP)

        def emit_pe_warm(n):
            for _ in range(n):
                wps = work.tile([128, 512], F32, tag="wk", name="wps")
                nc.tensor.matmul(
                    wps[0:64, :],
                    lhsT=warmpe[:, 0:64],
                    rhs=warmpe[:],
                    start=True,
                    stop=True,
                )

        # Q^T/K^T per head, duplicated across both partition halves
        qd = [persist.tile([128, S], F16, name=f"qd{h}") for h in range(3)]
        kd = [persist.tile([128, S], F16, name=f"kd{h}") for h in range(3)]
        # V: [k, 3*(64+1)] with a ones column per head (col 64 of each 65)
        v_sb = persist.tile([128, KT, 3 * 65], F16)
        for h in range(3):
            nc.vector.memset(
                v_sb[:].rearrange("p k (h x) -> p k h x", x=65)[:, :, h, 64:65], 1.0
            )
        # normalized context: heads 0,1 stacked; head 2 duplicated
        ctx01 = persist.tile([128, S], F16)
        ctx2d = persist.tile([128, S], F16)
        ctx_tmp = persist.tile([64, S], F16)

        # ---- PSUM: 3x [128,1024] work slots (12KB) + 2x [128,512] ctx
        # slots (4KB) = exactly 8 banks.
        ctx_pool = tc.alloc_tile_pool(name="ctx_ps", bufs=2, space="PSUM")
        work = tc.alloc_tile_pool(name="work", bufs=3, space="PSUM")
        p_pool = stack.enter_context(tc.tile_pool(name="p_sb", bufs=8))
        norm_pool = stack.enter_context(tc.tile_pool(name="norm", bufs=4))
        ob_pool = stack.enter_context(tc.tile_pool(name="ob", bufs=6))

        def emit_qk(w_sb, dst, b_sb, qt, bgroup, psum=None, dq=None):
            """One [128, 512] projection tile + bias + partition-dup DMAs."""
            dq = dq or nc.sync
            qs = slice(qt * 512, (qt + 1) * 512)
            pq = psum if psum is not None else work.tile(
                [128, 512], F32, tag="wk", name="pq"
            )
            for hc in range(HC):
                nc.tensor.matmul(
                    pq[:],
                    lhsT=w_sb[:, hc, :],
                    rhs=xt_sb[:, qt, hc, :],
                    start=(hc == 0),
                    stop=(hc == HC - 1),
                )
            if bgroup:
                # rows 0:64 = Q2, rows 64:128 = K2 (w_sb is [Wq2 | Wk2])
                nc.vector.tensor_scalar(
                    qd[2][0:64, qs], pq[0:64, :], b_sb[0:64, 1:2], None, ADD
                )
                nc.vector.tensor_scalar(
                    kd[2][64:128, qs], pq[64:128, :], b_sb[64:128, 1:2], None, ADD
                )
                dq.dma_start(qd[2][64:128, qs], qd[2][0:64, qs])
                dq.dma_start(kd[2][0:64, qs], kd[2][64:128, qs])
            else:
                nc.vector.tensor_scalar(
                    dst[0][0:64, qs], pq[0:64, :], b_sb[0:64, 0:1], None, ADD
                )
                nc.vector.tensor_scalar(
                    dst[1][64:128, qs], pq[64:128, :], b_sb[64:128, 0:1], None, ADD
                )
                dq.dma_start(dst[0][64:128, qs], dst[0][0:64, qs])
                dq.dma_start(dst[1][0:64, qs], dst[1][64:128, qs])

        def emit_v(kt):
            q, r = divmod(kt, 4)
            pv = work.tile([128, D3], F32, tag="wk", name="pv")
            for hc in range(HC):
                nc.tensor.matmul(
                    pv[:],
                    lhsT=xt_sb[:, q, hc, r * 128 : (r + 1) * 128],
                    rhs=wv_sb[:, hc, :],
                    start=(hc == 0),
                    stop=(hc == HC - 1),
                )
            nc.vector.tensor_tensor(
                v_sb[:].rearrange("p k (h x) -> p k h x", x=65)[:, kt, :, 0:64],
                pv[:].rearrange("p (h x) -> p h x", x=64),
                bv_bc[:].rearrange("p (h x) -> p h x", x=64),
                ADD,
            )

        # ---- score/exp/PV pipeline ----
        # pv_q holds exp'd tiles; each pop issues the 2 PV matmuls for one
        # [128,1024] prob tile. The lag keeps the in-order PE from stalling
        # on a not-yet-finished exp, and carries ctx WAR slack across
        # phase boundaries (ctx ring bufs=2).
        pv_q = []
        PV_LAG = 4

        def pop_pv(tail=False):
            h, half, c, cq, pt = pv_q.pop(0)
            for j in range(2):
                nc.tensor.matmul(
                    cq[j][:],
                    lhsT=v_sb[:, c, h * 65 : (h + 1) * 65],
                    rhs=pt[:, j * 512 : (j + 1) * 512],
                    start=(c == 0),
                    stop=(c == KT - 1),
                )
            # the final phase (h0, half1) normalizes per-quarter in the
            # tail instead; every other phase normalizes on its last pop
            if c == KT - 1 and not (h == 0 and half == 1):
                emit_normalize(h, half, cq)

        def emit_score_tile(h, half, c, cq, tail=False):
            """[128 keys, 1024 q] scores -> exp -> queue PV."""
            ks = slice(c * 128, (c + 1) * 128)
            sc = work.tile([128, 1024], F32, tag="wk", name="sc")
            for j in range(2):
                qs = slice(half * 1024 + j * 512, half * 1024 + (j + 1) * 512)
                nc.tensor.matmul(
                    sc[:, j * 512 : (j + 1) * 512],
                    lhsT=kd[h][:, ks],
                    rhs=qd[h][:, qs],
                    start=True,
                    stop=True,
                )
            pt = p_pool.tile([128, 1024], F16, tag="pt")
            nc.scalar.activation(
                pt[:], sc[:], EXP, bias=mask_sb[:, c : c + 1], scale=1.0
            )
            pv_q.append((h, half, c, cq, pt))
            # mostly-lazy drain: trim to 2 at the boundary and pop the two
            # leftovers in the next phase's first c's, so the normalize
            # chain (which gates the ctx-slot WAR for the NEXT phase's
            # first PV — the ctx ring is only 2 deep) starts early without
            # the PE ever waiting on a boundary exp that isn't done yet
            if c == KT - 1:
                while len(pv_q) > 2:
                    pop_pv(tail)
            elif c == 0:
                # pop the boundary leftovers now: their exps are (nearly)
                # done, and the previous phase's normalize then has ~4
                # tiles to clear the ctx-slot WAR before this phase's
                # first PV pops at c=4
                while len(pv_q) > 1:
                    pop_pv(tail)
            elif len(pv_q) > PV_LAG:
                pop_pv(tail)

        def emit_normalize(h, half, cq):
            """denom copy -> recip -> broadcast -> mult, stage-major over
            the half's two 512-col ctx quarters."""
            base = half * 1024
            if h == 0:
                dst = ctx01[0:64, :]
            elif h == 1:
                dst = ctx_tmp[:]
            else:
                dst = ctx2d[0:64, :]
            denoms, recips, rbcs = [], [], []
            for j in range(2):
                denom = norm_pool.tile([1, 512], F32, tag="denom")
                nc.vector.tensor_copy(denom[:], cq[j][64:65, :])
                denoms.append(denom)
            for j in range(2):
                recip = norm_pool.tile([1, 512], F32, tag="recip")
                nc.vector.reciprocal_approx_fast(recip[:], denoms[j][:])
                recips.append(recip)
            for j in range(2):
                rbc = norm_pool.tile([64, 512], F32, tag="rbc")
                nc.gpsimd.partition_broadcast(rbc[:], recips[j][:])
                rbcs.append(rbc)
            for j in range(2):
                dcols = slice(base + j * 512, base + (j + 1) * 512)
                nc.vector.tensor_tensor(
                    dst[:, dcols], cq[j][0:64, :], rbcs[j][:], MULT
                )
            if h == 1:
                nc.gpsimd.dma_start(
                    ctx01[64:128, base : base + 1024], ctx_tmp[:, base : base + 1024]
                )
            elif h == 2:
                nc.gpsimd.dma_start(
                    ctx2d[64:128, base : base + 1024], ctx2d[0:64, base : base + 1024]
                )

        def emit_norm_tail(half, cq):
            """Last-phase normalize, both quarters with minimal latency to
            the first mult: per-quarter copy+recip chains on DVE (ACT is
            still draining the final exps), broadcasts pipelined on GpSimd."""
            base = half * 1024
            rbcs = []
            for j in range(2):
                denom = norm_pool.tile([1, 512], F32, tag="denom")
                nc.vector.tensor_copy(denom[:], cq[j][64:65, :])
                recip = norm_pool.tile([1, 512], F32, tag="recip")
                nc.vector.reciprocal_approx_fast(recip[:], denom[:])
                rbc = norm_pool.tile([64, 512], F32, tag="rbc")
                nc.gpsimd.partition_broadcast(rbc[:], recip[:])
                rbcs.append(rbc)
            for j in range(2):
                dcols = slice(base + j * 512, base + (j + 1) * 512)
                nc.vector.tensor_tensor(
                    ctx01[0:64, dcols], cq[j][0:64, :], rbcs[j][:], MULT
                )

        def emit_outproj(qt, po, cast_eng, dma_eng):
            """po = ctx01^T Wo01 + ctx2d^T Wo2dup for one 128-row q-tile."""
            qs = slice(qt * 128, (qt + 1) * 128)
            # matmul dest is capped at 512 fp32 elements (one PSUM bank)
            for ns, ne in ((0, 512), (512, 768)):
                nc.tensor.matmul(
                    po[:, ns:ne],
                    lhsT=ctx01[:, qs],
                    rhs=wo_sb[:, ns:ne],
                    start=True,
                    stop=False,
                )
                nc.tensor.matmul(
                    po[:, ns:ne],
                    lhsT=ctx2d[:, qs],
                    rhs=wo2d[:, ns:ne],
                    start=False,
                    stop=True,
                )
            ob = ob_pool.tile([128, H], F16, tag="ob")
            if cast_eng == "v":
                nc.vector.tensor_copy(ob[:], po[:])
            else:
                nc.scalar.copy(ob[:], po[:])
            dma_eng.dma_start(out.ap()[qs, :], ob[:])

        # ---------------- emission schedule ----------------
        # Lead-in: dummy matmuls ramp the PE clock while the wb2/xt DMAs
        # land; head-2's Q/K come packed per wb2 tile (rows 0:64 = Q2,
        # 64:128 = K2), so two bgroup tiles cover all scores of phase 0's
        # first half. Gated on xt quarters 0/1 + the wb2 blob.
        emit_pe_warm(14)
        pb0 = work.tile([128, 512], F32, tag="wk", name="pb0")
        emit_qk(wb2_sb, None, bq_sb, 0, True, psum=pb0, dq=nc.scalar)
        pb1 = ctx_pool.tile([128, 512], F32, tag="ctx", name="pb1")
        emit_qk(wb2_sb, None, bq_sb, 1, True, psum=pb1, dq=nc.scalar)

        def ctx_quarters(name):
            return (
                ctx_pool.tile([65, 512], F32, tag="ctx", name=f"{name}q0"),
                ctx_pool.tile([65, 512], F32, tag="ctx", name=f"{name}q1"),
            )

        # fillers[phase][c] = list of thunks to emit before that score tile.
        # Phase-0 dups ride scalar (clear by ~12us, ACT still idle there);
        # phase-1/2 dups ride sync (clear by then; scalar's ACT engine is
        # busy with exps by phase 1 and a pseudo-DMA issue would wedge
        # ~600ns into the exp stream).
        def qk_thunk(w, dst, b, qt, bgroup=False, dq=None):
            return lambda: emit_qk(w, dst, b, qt, bgroup, dq=dq)

        fillers = {
            0: {
                5: [qk_thunk(wb2_sb, None, bq_sb, 2, True, dq=nc.scalar)],
                8: [qk_thunk(wk_sb, kd, bk_sb, 0, dq=nc.scalar)],
                9: [qk_thunk(wb2_sb, None, bq_sb, 3, True, dq=nc.scalar)],
                10: [qk_thunk(wq_sb, qd, bq_sb, 0, dq=nc.scalar)],
                # Q1 early enough that its bias+dup chain completes before
                # phase 1's first score needs qd[1]
                12: [qk_thunk(wq_sb, qd, bq_sb, 1, dq=nc.scalar)],
            },
            1: {
                1: [qk_thunk(wk_sb, kd, bk_sb, 1)],
                5: [qk_thunk(wk_sb, kd, bk_sb, 2)],
                9: [qk_thunk(wk_sb, kd, bk_sb, 3)],
            },
            2: {
                3: [qk_thunk(wq_sb, qd, bq_sb, 2)],
                7: [qk_thunk(wq_sb, qd, bq_sb, 3)],
            },
        }

        phases = [(2, 0), (1, 0), (0, 0), (2, 1), (1, 1), (0, 1)]

        def outproj_thunk(qt, cast_eng, dma_eng):
            def f():
                po = work.tile([128, H], F32, tag="wk", name=f"po{qt}")
                emit_outproj(qt, po, cast_eng, dma_eng)

            return f

        # outproj q0-7 spread over phases 3-5 (phases 4-5 are otherwise
        # ACT-paced with PE slack); casts ride DVE to keep ACT on exps
        # ops early in phases 4-5 are emitted BEFORE that phase's trailing
        # normalize of the previous phase, so they dodge the tile-granular
        # false dependency on the ctx tiles
        op_sched = {
            3: {11: 0, 13: 1, 14: 2},
            4: {1: 3, 2: 4, 13: 5},
            5: {1: 6, 2: 7},
        }
        for pi, (h, half) in enumerate(phases):
            tail = pi == 5
            cq = ctx_quarters(f"c{h}{half}")
            for c in range(KT):
                for thunk in fillers.get(pi, {}).get(c, []):
                    thunk()
                emit_score_tile(h, half, c, cq, tail)
                if pi == 0:
                    emit_v(c)
                qt_op = op_sched.get(pi, {}).get(c)
                if qt_op is not None:
                    outproj_thunk(
                        qt_op, "v", nc.sync if qt_op % 2 == 0 else nc.scalar
                    )()
            if tail:
                while pv_q:
                    pop_pv(tail=True)
                # normalize both quarters up front; dummy matmuls bridge the
                # PE gap while the chain runs so the clock stays at full
                # speed for the outprojs
                emit_norm_tail(half, cq)
                emit_pe_warm(16)
                engs = (nc.sync, nc.scalar, nc.gpsimd)
                for i, qt in enumerate(range(8, KT)):
                    po = work.tile([128, H], F32, tag="wk", name=f"po{qt}")
                    emit_outproj(qt, po, "s" if i % 2 else "v", engs[i % 3])

        work.release()
        ctx_pool.release()


_NC_CACHE = None


def _get_nc():
    global _NC_CACHE
    if _NC_CACHE is None:
        _NC_CACHE = build_kernel()
    return _NC_CACHE


def _pack_w(w, ncols):
    """[768, ncols] -> [128, HC*ncols] with row p = concat_c w[c*128+p, :]."""
    return np.ascontiguousarray(
        w.reshape(HC, 128, ncols).transpose(1, 0, 2).reshape(128, HC * ncols)
    )


def make_in_maps(hidden_states, attention_mask, Wq, bq, Wk, bk, Wv, bv, Wo, bo):
    hidden_states = np.asarray(hidden_states, np.float32)
    attention_mask = np.asarray(attention_mask, np.float32)
    Wq = np.asarray(Wq, np.float32)
    Wk = np.asarray(Wk, np.float32)
    Wv = np.asarray(Wv, np.float32)
    Wo = np.asarray(Wo, np.float32)
    bq = np.asarray(bq, np.float32)
    bk = np.asarray(bk, np.float32)
    bv = np.asarray(bv, np.float32)

    scale = 0.5 / np.sqrt(np.float32(HD))  # extra 1/2: scores use dup-row K=128
    in_maps = []
    for core in range(N_CORES):
        b, g = divmod(core, 4)
        cols = slice(D3 * g, D3 * (g + 1))
        bq_s = (bq[cols] * scale).astype(np.float32)
        bk_s = bk[cols].astype(np.float32)
        bq_pack = np.zeros((2, 128), np.float32)
        bq_pack[0] = bq_s[0:128]
        bq_pack[1, 0:64] = bq_s[128:192]
        bq_pack[1, 64:128] = bk_s[128:192]
        bk_pack = np.zeros((2, 128), np.float32)
        bk_pack[0] = bk_s[0:128]

        # xt s-quarter-major: [128, QT, HC, 512]
        xtp = (
            np.ascontiguousarray(hidden_states[b].T)
            .astype(np.float16)
            .reshape(HC, 128, QT, 512)
            .transpose(1, 2, 0, 3)
            .reshape(128, QT * HC * 512)
        )
        wq_p = _pack_w((Wq[:, cols][:, 0:128] * scale).astype(np.float16), 128)
        wk_p = _pack_w(Wk[:, cols][:, 0:128].astype(np.float16), 128)
        wb2_p = _pack_w(
            np.concatenate(
                [Wq[:, cols][:, 128:192] * scale, Wk[:, cols][:, 128:192]], axis=1
            ).astype(np.float16),
            128,
        )
        wqk_p = np.concatenate([wb2_p, wq_p, wk_p], axis=1)
        wv_p = _pack_w(Wv[:, cols].astype(np.float16), D3)
        wo2h = (Wo[cols, :][128:192] * 0.5).astype(np.float16)
        woo = np.concatenate(
            [
                Wo[cols, :][0:128].astype(np.float16),
                np.concatenate([wo2h, wo2h], axis=0),
            ],
            axis=1,
        )
        bqkm_p = np.concatenate(
            [
                bq_pack.T,
                bk_pack.T,
                attention_mask[b, 0, 0, :].reshape(KT, 128).T,
            ],
            axis=1,
        ).astype(np.float32)
        in_maps.append(
            {
                "xt": np.ascontiguousarray(xtp),
                "wqk": np.ascontiguousarray(wqk_p),
                "wv": np.ascontiguousarray(wv_p),
                "wo": np.ascontiguousarray(woo),
                "bqkm": np.ascontiguousarray(bqkm_p),
                "bv": bv[cols].reshape(1, D3).astype(np.float16),
            }
        )
    return in_maps


def assemble_out(results, bo):
    out = np.zeros((B, S, H), np.float32)
    for core in range(N_CORES):
        b = core // 4
        out[b] += results[core]["out"].astype(np.float32)
    out += np.asarray(bo, np.float32)
    return out


def kernel(hidden_states, attention_mask, Wq, bq, Wk, bk, Wv, bv, Wo, bo):
    in_maps = make_in_maps(
        hidden_states, attention_mask, Wq, bq, Wk, bk, Wv, bv, Wo, bo
    )
    res = run_bass_kernel_spmd(_get_nc(), in_maps, list(range(N_CORES)))
    return assemble_out(res.results, bo)
